# revision 24
# baseline (speedup 1.0000x reference)
"""Sliding-window GQA attention block (RoPE + QKV proj + SDPA + O proj) on 8
Trainium2 NeuronCores, head-sharded (1 kv-head group = 8 q-heads per core).

Contract: kernel(**inputs) takes the FULL unsharded inputs from
setup_inputs() and returns the FULL [1, 2048, 2880] output.

Per-core plan (core c owns q-heads [8c, 8c+8), kv-head c), all matmul
operands bf16:
  - QKV projections stream 512-wide bf16 matmuls (fp32 PSUM); RoPE in the
    PSUM epilogue (head-dim pre-permuted so rotate-half = partition-pair
    stream_shuffle, done in fp32 - bf16 shuffle is broken on HW). q lands in
    per-chunk tiles qc[c] [64, 8*512] (heads along free dim, order
    [0,2,4,6,1,3,5,7]), k in kT [64, S], v transposed to v_ext
    [seq, 64v + 64ones] tiles. x and wq stream via batched DMAs.
  - Attention per 128-query tile j, 4 heads per matmul (all 8 q-heads share
    the core's kv head): one [128,1024] PSUM tile per head-group holds
    scores vs key tiles j-1 | j; combined band-mask add (DVE), one exp
    (ACT, scale=1/8 folded) -> bf16 e tile. AV: lhsT = v_ext, rhs = e ->
    out.T [64 v + 64 dup-denominators, (h,q)]; denominators staged to SBUF
    (custom-DVE cannot read PSUM on HW), reciprocal_approx_fast, strided
    multiplies into per-tile aoj [128, 4*128] bf16.
  - Deep software pipeline: iteration j runs scores_j, AV_{j-1},
    oproj_{j-2} so no engine waits on a same-iteration producer.
  - O projection: 4x6 bf16 matmuls; PSUM chunks copied (ACT) into one
    [128, 2880] bf16 row tile, single DMA out per tile.
  - Host sums the 8 bf16 partials and adds wo_b.
"""
import sys

sys.path.insert(0, "/opt/trn_rl_repo")

import numpy as np

import concourse.bass as bass  # noqa: F401  (import keeps bass registered)
import concourse.tile as tile
from concourse import bacc, mybir
from concourse.bass_utils import run_bass_kernel_spmd

B, S, D = 1, 2048, 2880
H, KVH, HD = 64, 8, 64
WINDOW = 128
N_CORES = 8
DP = 2944  # padded contraction dim: 23 * 128 (2880 data + 1 ones row + pad)
KT = DP // 128  # 23 contraction tiles
NQT = S // 128  # 16 seq tiles
OCH = 480  # O-proj free chunk (6 * 480 = 2880)
WQG = [2, 7, 7, 7]  # wq k-tile DMA groups (small first group -> early PE start)
XH = [12, 11]  # x chunk half-tile k splits

F32 = mybir.dt.float32
BF16 = mybir.dt.bfloat16

# head order along q free dim: g0 = pair-firsts, g1 = pair-seconds
HEAD_ORDER = [0, 2, 4, 6, 1, 3, 5, 7]

# head-dim permutation: pairs (t, t+32) adjacent -> rotate-half partner is
# the neighbouring partition (stream_shuffle mask i^1 within quadrants)
PERM = np.empty(HD, dtype=np.int64)
PERM[0::2] = np.arange(32)
PERM[1::2] = np.arange(32) + 32

_COMPILED = None


def _build(debug=False):
    nc = bacc.Bacc("TRN2", target_bir_lowering=False, debug=False)

    xT_d = nc.dram_tensor("xT", [DP, S], BF16, kind="ExternalInput").ap()
    wq_d = nc.dram_tensor("wq", [DP, 512], BF16, kind="ExternalInput").ap()
    wkv_d = nc.dram_tensor("wkv", [DP, 128], BF16, kind="ExternalInput").ap()
    wo_d = nc.dram_tensor("wo", [512, D], BF16, kind="ExternalInput").ap()
    cos_d = nc.dram_tensor("cosT", [128, S], F32, kind="ExternalInput").ap()
    sin_d = nc.dram_tensor("sinTs", [128, S], F32, kind="ExternalInput").ap()
    maAB_d = nc.dram_tensor("maAB", [128, 1024], F32, kind="ExternalInput").ap()
    id_d = nc.dram_tensor("id64", [64, 64], BF16, kind="ExternalInput").ap()
    out_d = nc.dram_tensor("partial", [S, D], BF16, kind="ExternalOutput").ap()
    if debug:
        dbg_q_d = nc.dram_tensor("dbg_q", [64, 8 * S], BF16, kind="ExternalOutput").ap()
        dbg_k_d = nc.dram_tensor("dbg_k", [64, S], BF16, kind="ExternalOutput").ap()
        dbg_v_d = nc.dram_tensor("dbg_v", [64, S], BF16, kind="ExternalOutput").ap()
        dbg_e_d = nc.dram_tensor("dbg_e", [128, 2048], BF16, kind="ExternalOutput").ap()

    Exp = mybir.ActivationFunctionType.Exp
    SHUF_MASK = [i ^ 1 for i in range(32)]

    # DRAM views with the 128-partition dim explicit, for batched DMAs
    xT_v = xT_d.rearrange("(k p) s -> p k s", p=128)  # [128, 23, 2048]
    wq_v = wq_d.rearrange("(k p) m -> p k m", p=128)  # [128, 23, 512]
    wkv_v = wkv_d.rearrange("(k p) m -> p k m", p=128)  # [128, 23, 128]

    with tile.TileContext(nc) as tc:
        with (
            tc.tile_pool(name="constp", bufs=1) as constp,
            tc.tile_pool(name="qkvp", bufs=1) as qkvp,
            tc.tile_pool(name="vextp", bufs=1) as vextp,
            tc.tile_pool(name="workp", bufs=2) as workp,
        ):
            cos_t = constp.tile([128, S], F32)
            sin_t = constp.tile([128, S], F32)
            maAB_t = constp.tile([128, 1024], F32)
            id_t = constp.tile([64, 64], BF16)

            # persistent activations: q per chunk, k/v full
            qc = [qkvp.tile([64, 8 * 512], BF16, name=f"qc{c}") for c in range(4)]
            kT = qkvp.tile([64, S], BF16, name="kT")
            vT = qkvp.tile([64, S], BF16, name="vT")
            wo_sb = [qkvp.tile([128, D], BF16, name=f"wo{t}") for t in range(4)]
            v_ext = [vextp.tile([128, 128], BF16, name=f"vx{i}") for i in range(NQT)]

            # ---------------- Phase 1: QKV projections + RoPE ----------------
            with (
                tc.tile_pool(name="wpool", bufs=1) as wpool,
                tc.tile_pool(name="xsp", bufs=4) as xsp,
                tc.tile_pool(name="psq", bufs=6, space="PSUM") as psq,
            ):
                wq_g = [
                    wpool.tile([128, 512 * n], BF16, name=f"wqg{g}")
                    for g, n in enumerate(WQG)
                ]
                wkv_t = wpool.tile([128, 128 * KT], BF16, name="wkv_t")
                # x chunks as half-tiles (k 0:12 | 12:23), 4-buf rotation so the
                # next chunk's first half streams while the current one computes
                xh = [
                    [
                        xsp.tile([128, 512 * n], BF16, name="xh", tag="xh")
                        for n in XH
                    ]
                    for _ in range(4)
                ]

                def dma_x(sq, h, k0, k1):
                    base = 0 if h == 0 else XH[0]
                    nc.sync.dma_start(
                        xh[sq][h].rearrange("p (k s) -> p k s", s=512)[
                            :, k0 - base : k1 - base, :
                        ],
                        xT_v[:, k0:k1, 512 * sq : 512 * (sq + 1)],
                    )

                # startup order: small wq group + wkv + first x slice first
                k0 = 0
                x0_parts = [[(0, 2)], [(2, 9)], [(9, 12), (12, 16)], [(16, 23)]]
                for g, n in enumerate(WQG):
                    nc.sync.dma_start(
                        wq_g[g].rearrange("p (k m) -> p k m", m=512),
                        wq_v[:, k0 : k0 + n, :],
                    )
                    if g == 0:
                        nc.sync.dma_start(
                            wkv_t.rearrange("p (k m) -> p k m", m=128), wkv_v
                        )
                    for a, b in x0_parts[g]:
                        dma_x(0, 0 if a < XH[0] else 1, a, b)
                    k0 += n
                nc.sync.dma_start(cos_t[:], cos_d[:])
                nc.sync.dma_start(sin_t[:], sin_d[:])
                nc.sync.dma_start(maAB_t[:], maAB_d[:])
                nc.sync.dma_start(id_t[:], id_d[:])
                for t in range(4):
                    nc.sync.dma_start(wo_sb[t][:], wo_d[128 * t : 128 * (t + 1), :])
                for sq in range(1, 4):
                    dma_x(sq, 0, 0, XH[0])
                    dma_x(sq, 1, XH[0], KT)

                def wq_slice(k, mt):
                    g, i = 0, k
                    for n in WQG:
                        if i < n:
                            break
                        g, i = g + 1, i - n
                    c = 512 * i + 128 * mt
                    return wq_g[g][:, c : c + 128]

                def x_slice(sq, k):
                    h = 0 if k < XH[0] else 1
                    i = k if h == 0 else k - XH[0]
                    return xh[sq][h][:, 512 * i : 512 * (i + 1)]

                def rope_epilogue(sq, mt, ps):
                    c0 = 512 * sq
                    # fp32 through the shuffle (bf16 stream_shuffle is
                    # broken on TRN2 hardware); bf16 from the muls onward
                    t_all = workp.tile([128, 512], F32, tag="ra", name="t_all")
                    nc.scalar.copy(t_all[:], ps[:])
                    t_shuf = workp.tile([128, 512], F32, tag="rb", name="t_shuf")
                    nc.vector.stream_shuffle(t_shuf[:], t_all[:], SHUF_MASK)
                    t_cos = workp.tile([128, 512], BF16, tag="rc", name="t_cos")
                    nc.vector.tensor_mul(t_cos[:], t_all[:], cos_t[:, c0 : c0 + 512])
                    t_sin = workp.tile([128, 512], BF16, tag="rd", name="t_sin")
                    nc.vector.tensor_mul(t_sin[:], t_shuf[:], sin_t[:, c0 : c0 + 512])
                    # psum halves -> adjacent head blocks of this chunk's q
                    # tile (host packs wq columns in HEAD_ORDER)
                    b0 = 512 * (2 * mt)
                    b1 = 512 * (2 * mt + 1)
                    nc.vector.tensor_add(
                        qc[sq][:, b0 : b0 + 512], t_cos[0:64, :], t_sin[0:64, :]
                    )
                    nc.vector.tensor_add(
                        qc[sq][:, b1 : b1 + 512], t_cos[64:128, :], t_sin[64:128, :]
                    )

                for sq in range(4):
                    c0 = 512 * sq
                    psums = [
                        psq.tile([128, 512], F32, name="psq_t", tag="psq_t")
                        for _ in range(5)
                    ]
                    # sequential per-psum accumulation so each epilogue drains
                    # on ACT/DVE while the PE accumulates the next m-tile
                    for mt in range(5):
                        for k in range(KT):
                            nc.tensor.matmul(
                                psums[mt][:],
                                wq_slice(k, mt)
                                if mt < 4
                                else wkv_t[:, 128 * k : 128 * (k + 1)],
                                x_slice(sq, k),
                                start=(k == 0),
                                stop=(k == KT - 1),
                            )
                        if mt < 4:
                            rope_epilogue(sq, mt, psums[mt])
                    # kv epilogue: k rope (rows 0:64) + v copy (rows 64:128)
                    ps = psums[4]
                    t_allk = workp.tile([128, 512], F32, tag="ra", name="t_allk")
                    nc.scalar.copy(t_allk[0:64, :], ps[0:64, :])
                    t_shufk = workp.tile([128, 512], F32, tag="rb", name="t_shufk")
                    nc.vector.stream_shuffle(t_shufk[0:64, :], t_allk[0:64, :], SHUF_MASK)
                    t_cosk = workp.tile([128, 512], BF16, tag="rc", name="t_cosk")
                    nc.vector.tensor_mul(
                        t_cosk[0:64, :], t_allk[0:64, :], cos_t[0:64, c0 : c0 + 512]
                    )
                    t_sink = workp.tile([128, 512], BF16, tag="rd", name="t_sink")
                    nc.vector.tensor_mul(
                        t_sink[0:64, :], t_shufk[0:64, :], sin_t[0:64, c0 : c0 + 512]
                    )
                    nc.vector.tensor_add(
                        kT[:, c0 : c0 + 512], t_cosk[0:64, :], t_sink[0:64, :]
                    )
                    nc.vector.tensor_copy(vT[:, c0 : c0 + 512], ps[64:128, :])
                    # v transposes for this chunk's 4 seq tiles
                    for i in range(4 * sq, 4 * sq + 4):
                        tr = psq.tile([128, 64], BF16, name="vtr", tag="vtr", bufs=2)
                        nc.tensor.transpose(tr[:], vT[:, 128 * i : 128 * (i + 1)], id_t[:])
                        nc.vector.tensor_copy(v_ext[i][:, 0:64], tr[:])
                        nc.gpsimd.memset(v_ext[i][:, 64:128], 1.0)

            # ------------- Phase 2: attention + O-projection, pipelined -------
            with (
                tc.tile_pool(name="epool", bufs=6) as epool,
                tc.tile_pool(name="recp", bufs=2) as recp,
                tc.tile_pool(name="aop", bufs=4) as aop,
                tc.tile_pool(name="outsp", bufs=2) as outsp,
                tc.tile_pool(name="psS", bufs=2, space="PSUM") as psS,
                tc.tile_pool(name="psAV", bufs=1, space="PSUM") as psAV,
                tc.tile_pool(name="psP", bufs=2, space="PSUM") as psP,
            ):
                e_hist = {}
                ao_hist = {}

                def scores(j):
                    """Scores + mask + exp for tile j -> e_hist[j]."""
                    cqc = qc[j // 4].rearrange("p (b s) -> p b s", s=512)
                    jo = 128 * (j % 4)
                    e_g = []
                    for g in range(2):
                        sc = psS.tile([128, 1024], F32, name="sc", tag="ps_s")
                        qslc = cqc[:, 4 * g : 4 * g + 4, jo : jo + 128]
                        if j > 0:
                            nc.tensor.matmul(
                                sc[:, 0:512],
                                kT[:, 128 * (j - 1) : 128 * j],
                                qslc,
                                start=True,
                                stop=True,
                            )
                        nc.tensor.matmul(
                            sc[:, 512:1024],
                            kT[:, 128 * j : 128 * (j + 1)],
                            qslc,
                            start=True,
                            stop=True,
                        )
                        e_t = epool.tile([128, 1024], BF16, tag="e", name="e_t")
                        if j > 0:
                            nc.vector.tensor_add(sc[:], sc[:], maAB_t[:])
                            nc.scalar.activation(e_t[:], sc[:], Exp, scale=0.125)
                        else:
                            nc.vector.tensor_add(
                                sc[:, 512:1024], sc[:, 512:1024], maAB_t[:, 512:1024]
                            )
                            nc.scalar.activation(
                                e_t[:, 512:1024], sc[:, 512:1024], Exp, scale=0.125
                            )
                        e_g.append(e_t)
                    e_hist[j] = e_g

                def av_norm(j):
                    """AV + denominators + normalize for tile j -> ao_hist[j]."""
                    e_g = e_hist.pop(j)
                    pav = psAV.tile([128, 1024], F32, name="pav", tag="pav")
                    for g in range(2):
                        if j > 0:
                            nc.tensor.matmul(
                                pav[:, 512 * g : 512 * (g + 1)],
                                v_ext[j - 1][:],
                                e_g[g][:, 0:512],
                                start=True,
                                stop=False,
                            )
                        nc.tensor.matmul(
                            pav[:, 512 * g : 512 * (g + 1)],
                            v_ext[j][:],
                            e_g[g][:, 512:1024],
                            start=(j == 0),
                            stop=True,
                        )
                    # custom-DVE ops cannot read PSUM on HW: stage via ACT
                    den = recp.tile([64, 1024], F32, name="den", tag="den")
                    nc.scalar.copy(den[:], pav[64:128, :])
                    rec = recp.tile([64, 1024], F32, name="rec", tag="rec")
                    nc.vector.reciprocal_approx_fast(rec[:], den[:])
                    aoj = aop.tile([128, 512], BF16, tag="ao", name="aoj")
                    aov = aoj.rearrange("p (t s) -> p t s", t=4)
                    for g in range(2):
                        nc.vector.tensor_mul(
                            aov[64 * g : 64 * (g + 1), :, :],
                            pav[0:64, 512 * g : 512 * (g + 1)].rearrange(
                                "p (h q) -> p h q", h=4
                            ),
                            rec[:, 512 * g : 512 * (g + 1)].rearrange(
                                "p (h q) -> p h q", h=4
                            ),
                        )
                    ao_hist[j] = aoj

                def oproj(j):
                    aoj = ao_hist.pop(j)
                    out_row = outsp.tile([128, D], BF16, tag="orow", name="orow")
                    for ch in range(6):
                        pp = psP.tile([128, OCH], F32, name="pp", tag="pp")
                        for t in range(4):
                            nc.tensor.matmul(
                                pp[:],
                                aoj[:, 128 * t : 128 * (t + 1)],
                                wo_sb[t][:, OCH * ch : OCH * (ch + 1)],
                                start=(t == 0),
                                stop=(t == 3),
                            )
                        nc.scalar.copy(out_row[:, OCH * ch : OCH * (ch + 1)], pp[:])
                    nc.sync.dma_start(out_d[128 * j : 128 * (j + 1), :], out_row[:])

                for j in range(NQT):
                    scores(j)
                    if j >= 1:
                        av_norm(j - 1)
                    if j >= 2:
                        oproj(j - 2)
                    if debug and j == 2:
                        nc.sync.dma_start(dbg_e_d[:, 0:1024], e_hist[2][0][:])
                        nc.sync.dma_start(dbg_e_d[:, 1024:2048], e_hist[2][1][:])
                av_norm(NQT - 1)
                oproj(NQT - 2)
                oproj(NQT - 1)
                if debug:
                    for c in range(4):
                        nc.sync.dma_start(
                            dbg_q_d[:, 4096 * c : 4096 * (c + 1)], qc[c][:]
                        )
                    nc.sync.dma_start(dbg_k_d[:], kT[:])
                    nc.sync.dma_start(dbg_v_d[:], vT[:])
    nc.compile()
    return nc


def _prep_inputs(x, rope_cache, wq_w, wq_b, wk_w, wk_b, wv_w, wv_b, wo_w):
    """Build the shared + per-core input maps."""
    import ml_dtypes

    bf16 = ml_dtypes.bfloat16

    xT = np.zeros((DP, S), dtype=np.float32)
    xT[0:D, :] = np.ascontiguousarray(x[0].T)
    xT[D, :] = 1.0  # bias row

    cos = np.asarray(rope_cache[:, 0, :], dtype=np.float32)  # [S, 64]
    sin = np.asarray(rope_cache[:, 1, :], dtype=np.float32)
    cosP = cos[:, PERM].T  # [64, S] permuted head-dim rows
    sinP = sin[:, PERM].T
    sign = np.where(PERM < 32, -1.0, 1.0).astype(np.float32)[:, None]
    sinPs = sinP * sign
    cosT = np.concatenate([cosP, cosP], axis=0).astype(np.float32)  # [128, S]
    sinTs = np.concatenate([sinPs, sinPs], axis=0).astype(np.float32)

    kk = np.arange(128)[:, None]
    qq = np.arange(128)[None, :]
    maB1 = np.where(kk <= qq, 0.0, -1e30).astype(np.float32)  # same-tile causal
    maA1 = np.where(qq < kk, 0.0, -1e30).astype(np.float32)  # prev-tile window
    maAB = np.concatenate([np.tile(maA1, (1, 4)), np.tile(maB1, (1, 4))], axis=1)

    id64 = np.eye(64, dtype=np.float32).astype(bf16)

    shared = dict(
        xT=xT.astype(bf16),
        cosT=cosT,
        sinTs=sinTs,
        maAB=maAB,
        id64=id64,
    )

    in_maps = []
    for c in range(N_CORES):
        # wq slice: q heads [8c, 8c+8) in block order HEAD_ORDER, head-dim
        # permuted, transposed, bias row
        wq_rows = []
        bq_rows = []
        for lh in HEAD_ORDER:
            g = 8 * c + lh
            wq_rows.append(wq_w[64 * g + PERM, :])  # [64, D]
            bq_rows.append(wq_b[64 * g + PERM])
        wq_slice = np.concatenate(wq_rows, axis=0)  # [512, D]
        bq_slice = np.concatenate(bq_rows, axis=0)  # [512]
        wq_t = np.zeros((DP, 512), dtype=np.float32)
        wq_t[0:D, :] = wq_slice.T
        wq_t[D, :] = bq_slice

        wk_slice = wk_w[64 * c + PERM, :]  # [64, D] permuted
        bk_slice = wk_b[64 * c + PERM]
        wv_slice = wv_w[64 * c : 64 * (c + 1), :]  # unpermuted
        bv_slice = wv_b[64 * c : 64 * (c + 1)]
        wkv_t = np.zeros((DP, 128), dtype=np.float32)
        wkv_t[0:D, 0:64] = wk_slice.T
        wkv_t[0:D, 64:128] = wv_slice.T
        wkv_t[D, 0:64] = bk_slice
        wkv_t[D, 64:128] = bv_slice

        wo_t = np.ascontiguousarray(wo_w[:, 512 * c : 512 * (c + 1)].T)  # [512, D]

        in_maps.append(
            dict(
                shared,
                wq=wq_t.astype(bf16),
                wkv=wkv_t.astype(bf16),
                wo=wo_t.astype(bf16),
            )
        )
    return in_maps


def _run(inputs, trace):
    global _COMPILED
    if _COMPILED is None:
        _COMPILED = _build()
    args = [
        np.asarray(inputs[k], dtype=np.float32)
        for k in (
            "x",
            "rope_cache",
            "wq_w",
            "wq_b",
            "wk_w",
            "wk_b",
            "wv_w",
            "wv_b",
            "wo_w",
        )
    ]
    in_maps = _prep_inputs(*args)
    res = run_bass_kernel_spmd(
        _COMPILED, in_maps, core_ids=list(range(N_CORES)), trace=trace
    )
    out = np.zeros((S, D), dtype=np.float32)
    for c in range(N_CORES):
        out += res.results[c]["partial"]
    out += np.asarray(inputs["wo_b"], np.float32)[None, :]
    return out.reshape(B, S, D).astype(np.float32), res


def kernel(**inputs):
    out, _ = _run(inputs, trace=False)
    return out


# expose the compiled-module runner for test harnesses that want tracing
def run_traced(**inputs):
    return _run(inputs, trace=True)


# revision 25
# speedup vs baseline: 1.0067x; 1.0067x over previous
"""Sliding-window GQA attention block (RoPE + QKV proj + SDPA + O proj) on 8
Trainium2 NeuronCores, head-sharded (1 kv-head group = 8 q-heads per core).

Contract: kernel(**inputs) takes the FULL unsharded inputs from
setup_inputs() and returns the FULL [1, 2048, 2880] output.

Per-core plan (core c owns q-heads [8c, 8c+8), kv-head c), all matmul
operands bf16:
  - QKV projections stream 512-wide bf16 matmuls (fp32 PSUM); RoPE in the
    PSUM epilogue (head-dim pre-permuted so rotate-half = partition-pair
    stream_shuffle, done in fp32 - bf16 shuffle is broken on HW). q lands in
    per-chunk tiles qc[c] [64, 8*512] (heads along free dim, order
    [0,2,4,6,1,3,5,7]), k in kT [64, S], v transposed to v_ext
    [seq, 64v + 64ones] tiles. x and wq stream via batched DMAs.
  - Attention per 128-query tile j, 4 heads per matmul (all 8 q-heads share
    the core's kv head): one [128,1024] PSUM tile per head-group holds
    scores vs key tiles j-1 | j; combined band-mask add (DVE), one exp
    (ACT, scale=1/8 folded) -> bf16 e tile. AV: lhsT = v_ext, rhs = e ->
    out.T [64 v + 64 dup-denominators, (h,q)]; denominators staged to SBUF
    (custom-DVE cannot read PSUM on HW), reciprocal_approx_fast, strided
    multiplies into per-tile aoj [128, 4*128] bf16.
  - Deep software pipeline: iteration j runs scores_j, AV_{j-1},
    oproj_{j-2} so no engine waits on a same-iteration producer.
  - O projection: 4x6 bf16 matmuls; PSUM chunks copied (ACT) into one
    [128, 2880] bf16 row tile, single DMA out per tile.
  - Host sums the 8 bf16 partials and adds wo_b.
"""
import sys

sys.path.insert(0, "/opt/trn_rl_repo")

import numpy as np

import concourse.bass as bass  # noqa: F401  (import keeps bass registered)
import concourse.tile as tile
from concourse import bacc, mybir
from concourse.bass_utils import run_bass_kernel_spmd

B, S, D = 1, 2048, 2880
H, KVH, HD = 64, 8, 64
WINDOW = 128
N_CORES = 8
DP = 2944  # padded contraction dim: 23 * 128 (2880 data + 1 ones row + pad)
KT = DP // 128  # 23 contraction tiles
NQT = S // 128  # 16 seq tiles
OCH = 480  # O-proj free chunk (6 * 480 = 2880)
WQG = [2, 7, 7, 7]  # wq k-tile DMA groups (small first group -> early PE start)
XH = [12, 11]  # x chunk half-tile k splits

F32 = mybir.dt.float32
BF16 = mybir.dt.bfloat16

# head order along q free dim: g0 = pair-firsts, g1 = pair-seconds
HEAD_ORDER = [0, 2, 4, 6, 1, 3, 5, 7]

# head-dim permutation: pairs (t, t+32) adjacent -> rotate-half partner is
# the neighbouring partition (stream_shuffle mask i^1 within quadrants)
PERM = np.empty(HD, dtype=np.int64)
PERM[0::2] = np.arange(32)
PERM[1::2] = np.arange(32) + 32

_COMPILED = None


def _build(debug=False):
    nc = bacc.Bacc("TRN2", target_bir_lowering=False, debug=False)

    xT_d = nc.dram_tensor("xT", [DP, S], BF16, kind="ExternalInput").ap()
    wq_d = nc.dram_tensor("wq", [DP, 512], BF16, kind="ExternalInput").ap()
    wkv_d = nc.dram_tensor("wkv", [DP, 128], BF16, kind="ExternalInput").ap()
    wo_d = nc.dram_tensor("wo", [512, D], BF16, kind="ExternalInput").ap()
    cos_d = nc.dram_tensor("cosT", [128, S], F32, kind="ExternalInput").ap()
    sin_d = nc.dram_tensor("sinTs", [128, S], F32, kind="ExternalInput").ap()
    maAB_d = nc.dram_tensor("maAB", [128, 1024], F32, kind="ExternalInput").ap()
    id_d = nc.dram_tensor("id64", [64, 64], BF16, kind="ExternalInput").ap()
    out_d = nc.dram_tensor("partial", [S, D], BF16, kind="ExternalOutput").ap()
    if debug:
        dbg_q_d = nc.dram_tensor("dbg_q", [64, 8 * S], BF16, kind="ExternalOutput").ap()
        dbg_k_d = nc.dram_tensor("dbg_k", [64, S], BF16, kind="ExternalOutput").ap()
        dbg_v_d = nc.dram_tensor("dbg_v", [64, S], BF16, kind="ExternalOutput").ap()
        dbg_e_d = nc.dram_tensor("dbg_e", [128, 2048], BF16, kind="ExternalOutput").ap()

    Exp = mybir.ActivationFunctionType.Exp
    SHUF_MASK = [i ^ 1 for i in range(32)]

    # DRAM views with the 128-partition dim explicit, for batched DMAs
    xT_v = xT_d.rearrange("(k p) s -> p k s", p=128)  # [128, 23, 2048]
    wq_v = wq_d.rearrange("(k p) m -> p k m", p=128)  # [128, 23, 512]
    wkv_v = wkv_d.rearrange("(k p) m -> p k m", p=128)  # [128, 23, 128]

    with tile.TileContext(nc) as tc:
        with (
            tc.tile_pool(name="constp", bufs=1) as constp,
            tc.tile_pool(name="qkvp", bufs=1) as qkvp,
            tc.tile_pool(name="vextp", bufs=1) as vextp,
            tc.tile_pool(name="workp", bufs=2) as workp,
        ):
            cos_t = constp.tile([128, S], F32)
            sin_t = constp.tile([128, S], F32)
            maAB_t = constp.tile([128, 1024], F32)
            id_t = constp.tile([64, 64], BF16)

            # persistent activations: q per chunk, k/v full
            qc = [qkvp.tile([64, 8 * 512], BF16, name=f"qc{c}") for c in range(4)]
            kT = qkvp.tile([64, S], BF16, name="kT")
            vT = qkvp.tile([64, S], BF16, name="vT")
            wo_sb = [qkvp.tile([128, D], BF16, name=f"wo{t}") for t in range(4)]
            v_ext = [vextp.tile([128, 128], BF16, name=f"vx{i}") for i in range(NQT)]

            # ---------------- Phase 1: QKV projections + RoPE ----------------
            with (
                tc.tile_pool(name="wpool", bufs=1) as wpool,
                tc.tile_pool(name="xsp", bufs=4) as xsp,
                tc.tile_pool(name="psq", bufs=6, space="PSUM") as psq,
            ):
                wq_g = [
                    wpool.tile([128, 512 * n], BF16, name=f"wqg{g}")
                    for g, n in enumerate(WQG)
                ]
                wkv_t = wpool.tile([128, 128 * KT], BF16, name="wkv_t")
                # x chunks as half-tiles (k 0:12 | 12:23), 4-buf rotation so the
                # next chunk's first half streams while the current one computes
                xh = [
                    [
                        xsp.tile([128, 512 * n], BF16, name="xh", tag="xh")
                        for n in XH
                    ]
                    for _ in range(4)
                ]

                def dma_x(sq, h, k0, k1):
                    base = 0 if h == 0 else XH[0]
                    nc.sync.dma_start(
                        xh[sq][h].rearrange("p (k s) -> p k s", s=512)[
                            :, k0 - base : k1 - base, :
                        ],
                        xT_v[:, k0:k1, 512 * sq : 512 * (sq + 1)],
                    )

                # startup order: small wq group + wkv + first x slice first
                k0 = 0
                x0_parts = [[(0, 2)], [(2, 9)], [(9, 12), (12, 16)], [(16, 23)]]
                for g, n in enumerate(WQG):
                    nc.sync.dma_start(
                        wq_g[g].rearrange("p (k m) -> p k m", m=512),
                        wq_v[:, k0 : k0 + n, :],
                    )
                    if g == 0:
                        nc.sync.dma_start(
                            wkv_t.rearrange("p (k m) -> p k m", m=128), wkv_v
                        )
                    for a, b in x0_parts[g]:
                        dma_x(0, 0 if a < XH[0] else 1, a, b)
                    k0 += n
                nc.sync.dma_start(cos_t[:], cos_d[:])
                nc.sync.dma_start(sin_t[:], sin_d[:])
                nc.sync.dma_start(maAB_t[:], maAB_d[:])
                nc.sync.dma_start(id_t[:], id_d[:])
                for t in range(4):
                    nc.sync.dma_start(wo_sb[t][:], wo_d[128 * t : 128 * (t + 1), :])
                for sq in range(1, 4):
                    dma_x(sq, 0, 0, XH[0])
                    dma_x(sq, 1, XH[0], KT)

                def wq_slice(k, mt):
                    g, i = 0, k
                    for n in WQG:
                        if i < n:
                            break
                        g, i = g + 1, i - n
                    c = 512 * i + 128 * mt
                    return wq_g[g][:, c : c + 128]

                def x_slice(sq, k):
                    h = 0 if k < XH[0] else 1
                    i = k if h == 0 else k - XH[0]
                    return xh[sq][h][:, 512 * i : 512 * (i + 1)]

                def rope_epilogue(sq, mt, ps):
                    c0 = 512 * sq
                    # fp32 through the shuffle (bf16 stream_shuffle is
                    # broken on TRN2 hardware); bf16 from the muls onward
                    t_all = workp.tile([128, 512], F32, tag="ra", name="t_all")
                    nc.scalar.copy(t_all[:], ps[:])
                    t_shuf = workp.tile([128, 512], F32, tag="rb", name="t_shuf")
                    nc.vector.stream_shuffle(t_shuf[:], t_all[:], SHUF_MASK)
                    t_cos = workp.tile([128, 512], BF16, tag="rc", name="t_cos")
                    nc.vector.tensor_mul(t_cos[:], t_all[:], cos_t[:, c0 : c0 + 512])
                    t_sin = workp.tile([128, 512], BF16, tag="rd", name="t_sin")
                    nc.vector.tensor_mul(t_sin[:], t_shuf[:], sin_t[:, c0 : c0 + 512])
                    # psum halves -> adjacent head blocks of this chunk's q
                    # tile (host packs wq columns in HEAD_ORDER)
                    b0 = 512 * (2 * mt)
                    b1 = 512 * (2 * mt + 1)
                    nc.vector.tensor_add(
                        qc[sq][:, b0 : b0 + 512], t_cos[0:64, :], t_sin[0:64, :]
                    )
                    nc.vector.tensor_add(
                        qc[sq][:, b1 : b1 + 512], t_cos[64:128, :], t_sin[64:128, :]
                    )

                for sq in range(4):
                    c0 = 512 * sq
                    psums = [
                        psq.tile([128, 512], F32, name="psq_t", tag="psq_t")
                        for _ in range(5)
                    ]
                    # pairwise-interleaved accumulation: same-bank matmuls stay
                    # one apart (avoids the accumulation hazard) while early
                    # m-tiles finish mid-chunk so their epilogues drain on
                    # ACT/DVE during the rest of the accumulation
                    for grp in ((0, 1), (2, 3), (4,)):
                        for k in range(KT):
                            for mt in grp:
                                nc.tensor.matmul(
                                    psums[mt][:],
                                    wq_slice(k, mt)
                                    if mt < 4
                                    else wkv_t[:, 128 * k : 128 * (k + 1)],
                                    x_slice(sq, k),
                                    start=(k == 0),
                                    stop=(k == KT - 1),
                                )
                        for mt in grp:
                            if mt < 4:
                                rope_epilogue(sq, mt, psums[mt])
                    # kv epilogue: k rope (rows 0:64) + v copy (rows 64:128)
                    ps = psums[4]
                    t_allk = workp.tile([128, 512], F32, tag="ra", name="t_allk")
                    nc.scalar.copy(t_allk[0:64, :], ps[0:64, :])
                    t_shufk = workp.tile([128, 512], F32, tag="rb", name="t_shufk")
                    nc.vector.stream_shuffle(t_shufk[0:64, :], t_allk[0:64, :], SHUF_MASK)
                    t_cosk = workp.tile([128, 512], BF16, tag="rc", name="t_cosk")
                    nc.vector.tensor_mul(
                        t_cosk[0:64, :], t_allk[0:64, :], cos_t[0:64, c0 : c0 + 512]
                    )
                    t_sink = workp.tile([128, 512], BF16, tag="rd", name="t_sink")
                    nc.vector.tensor_mul(
                        t_sink[0:64, :], t_shufk[0:64, :], sin_t[0:64, c0 : c0 + 512]
                    )
                    nc.vector.tensor_add(
                        kT[:, c0 : c0 + 512], t_cosk[0:64, :], t_sink[0:64, :]
                    )
                    nc.vector.tensor_copy(vT[:, c0 : c0 + 512], ps[64:128, :])
                    # v transposes for this chunk's 4 seq tiles
                    for i in range(4 * sq, 4 * sq + 4):
                        tr = psq.tile([128, 64], BF16, name="vtr", tag="vtr", bufs=2)
                        nc.tensor.transpose(tr[:], vT[:, 128 * i : 128 * (i + 1)], id_t[:])
                        nc.vector.tensor_copy(v_ext[i][:, 0:64], tr[:])
                        nc.gpsimd.memset(v_ext[i][:, 64:128], 1.0)

            # ------------- Phase 2: attention + O-projection, pipelined -------
            with (
                tc.tile_pool(name="epool", bufs=6) as epool,
                tc.tile_pool(name="recp", bufs=2) as recp,
                tc.tile_pool(name="aop", bufs=4) as aop,
                tc.tile_pool(name="outsp", bufs=2) as outsp,
                tc.tile_pool(name="psS", bufs=2, space="PSUM") as psS,
                tc.tile_pool(name="psAV", bufs=1, space="PSUM") as psAV,
                tc.tile_pool(name="psP", bufs=2, space="PSUM") as psP,
            ):
                e_hist = {}
                ao_hist = {}

                def scores(j):
                    """Scores + mask + exp for tile j -> e_hist[j]."""
                    cqc = qc[j // 4].rearrange("p (b s) -> p b s", s=512)
                    jo = 128 * (j % 4)
                    e_g = []
                    for g in range(2):
                        sc = psS.tile([128, 1024], F32, name="sc", tag="ps_s")
                        qslc = cqc[:, 4 * g : 4 * g + 4, jo : jo + 128]
                        if j > 0:
                            nc.tensor.matmul(
                                sc[:, 0:512],
                                kT[:, 128 * (j - 1) : 128 * j],
                                qslc,
                                start=True,
                                stop=True,
                            )
                        nc.tensor.matmul(
                            sc[:, 512:1024],
                            kT[:, 128 * j : 128 * (j + 1)],
                            qslc,
                            start=True,
                            stop=True,
                        )
                        e_t = epool.tile([128, 1024], BF16, tag="e", name="e_t")
                        if j > 0:
                            nc.vector.tensor_add(sc[:], sc[:], maAB_t[:])
                            nc.scalar.activation(e_t[:], sc[:], Exp, scale=0.125)
                        else:
                            nc.vector.tensor_add(
                                sc[:, 512:1024], sc[:, 512:1024], maAB_t[:, 512:1024]
                            )
                            nc.scalar.activation(
                                e_t[:, 512:1024], sc[:, 512:1024], Exp, scale=0.125
                            )
                        e_g.append(e_t)
                    e_hist[j] = e_g

                def av_norm(j):
                    """AV + denominators + normalize for tile j -> ao_hist[j]."""
                    e_g = e_hist.pop(j)
                    pav = psAV.tile([128, 1024], F32, name="pav", tag="pav")
                    for g in range(2):
                        if j > 0:
                            nc.tensor.matmul(
                                pav[:, 512 * g : 512 * (g + 1)],
                                v_ext[j - 1][:],
                                e_g[g][:, 0:512],
                                start=True,
                                stop=False,
                            )
                        nc.tensor.matmul(
                            pav[:, 512 * g : 512 * (g + 1)],
                            v_ext[j][:],
                            e_g[g][:, 512:1024],
                            start=(j == 0),
                            stop=True,
                        )
                    # custom-DVE ops cannot read PSUM on HW: stage via ACT
                    den = recp.tile([64, 1024], F32, name="den", tag="den")
                    nc.scalar.copy(den[:], pav[64:128, :])
                    rec = recp.tile([64, 1024], F32, name="rec", tag="rec")
                    nc.vector.reciprocal_approx_fast(rec[:], den[:])
                    aoj = aop.tile([128, 512], BF16, tag="ao", name="aoj")
                    aov = aoj.rearrange("p (t s) -> p t s", t=4)
                    for g in range(2):
                        nc.vector.tensor_mul(
                            aov[64 * g : 64 * (g + 1), :, :],
                            pav[0:64, 512 * g : 512 * (g + 1)].rearrange(
                                "p (h q) -> p h q", h=4
                            ),
                            rec[:, 512 * g : 512 * (g + 1)].rearrange(
                                "p (h q) -> p h q", h=4
                            ),
                        )
                    ao_hist[j] = aoj

                def oproj(j):
                    aoj = ao_hist.pop(j)
                    out_row = outsp.tile([128, D], BF16, tag="orow", name="orow")
                    for ch in range(6):
                        pp = psP.tile([128, OCH], F32, name="pp", tag="pp")
                        for t in range(4):
                            nc.tensor.matmul(
                                pp[:],
                                aoj[:, 128 * t : 128 * (t + 1)],
                                wo_sb[t][:, OCH * ch : OCH * (ch + 1)],
                                start=(t == 0),
                                stop=(t == 3),
                            )
                        nc.scalar.copy(out_row[:, OCH * ch : OCH * (ch + 1)], pp[:])
                    nc.sync.dma_start(out_d[128 * j : 128 * (j + 1), :], out_row[:])

                for j in range(NQT):
                    scores(j)
                    if j >= 1:
                        av_norm(j - 1)
                    if j >= 2:
                        oproj(j - 2)
                    if debug and j == 2:
                        nc.sync.dma_start(dbg_e_d[:, 0:1024], e_hist[2][0][:])
                        nc.sync.dma_start(dbg_e_d[:, 1024:2048], e_hist[2][1][:])
                av_norm(NQT - 1)
                oproj(NQT - 2)
                oproj(NQT - 1)
                if debug:
                    for c in range(4):
                        nc.sync.dma_start(
                            dbg_q_d[:, 4096 * c : 4096 * (c + 1)], qc[c][:]
                        )
                    nc.sync.dma_start(dbg_k_d[:], kT[:])
                    nc.sync.dma_start(dbg_v_d[:], vT[:])
    nc.compile()
    return nc


def _prep_inputs(x, rope_cache, wq_w, wq_b, wk_w, wk_b, wv_w, wv_b, wo_w):
    """Build the shared + per-core input maps."""
    import ml_dtypes

    bf16 = ml_dtypes.bfloat16

    xT = np.zeros((DP, S), dtype=np.float32)
    xT[0:D, :] = np.ascontiguousarray(x[0].T)
    xT[D, :] = 1.0  # bias row

    cos = np.asarray(rope_cache[:, 0, :], dtype=np.float32)  # [S, 64]
    sin = np.asarray(rope_cache[:, 1, :], dtype=np.float32)
    cosP = cos[:, PERM].T  # [64, S] permuted head-dim rows
    sinP = sin[:, PERM].T
    sign = np.where(PERM < 32, -1.0, 1.0).astype(np.float32)[:, None]
    sinPs = sinP * sign
    cosT = np.concatenate([cosP, cosP], axis=0).astype(np.float32)  # [128, S]
    sinTs = np.concatenate([sinPs, sinPs], axis=0).astype(np.float32)

    kk = np.arange(128)[:, None]
    qq = np.arange(128)[None, :]
    maB1 = np.where(kk <= qq, 0.0, -1e30).astype(np.float32)  # same-tile causal
    maA1 = np.where(qq < kk, 0.0, -1e30).astype(np.float32)  # prev-tile window
    maAB = np.concatenate([np.tile(maA1, (1, 4)), np.tile(maB1, (1, 4))], axis=1)

    id64 = np.eye(64, dtype=np.float32).astype(bf16)

    shared = dict(
        xT=xT.astype(bf16),
        cosT=cosT,
        sinTs=sinTs,
        maAB=maAB,
        id64=id64,
    )

    in_maps = []
    for c in range(N_CORES):
        # wq slice: q heads [8c, 8c+8) in block order HEAD_ORDER, head-dim
        # permuted, transposed, bias row
        wq_rows = []
        bq_rows = []
        for lh in HEAD_ORDER:
            g = 8 * c + lh
            wq_rows.append(wq_w[64 * g + PERM, :])  # [64, D]
            bq_rows.append(wq_b[64 * g + PERM])
        wq_slice = np.concatenate(wq_rows, axis=0)  # [512, D]
        bq_slice = np.concatenate(bq_rows, axis=0)  # [512]
        wq_t = np.zeros((DP, 512), dtype=np.float32)
        wq_t[0:D, :] = wq_slice.T
        wq_t[D, :] = bq_slice

        wk_slice = wk_w[64 * c + PERM, :]  # [64, D] permuted
        bk_slice = wk_b[64 * c + PERM]
        wv_slice = wv_w[64 * c : 64 * (c + 1), :]  # unpermuted
        bv_slice = wv_b[64 * c : 64 * (c + 1)]
        wkv_t = np.zeros((DP, 128), dtype=np.float32)
        wkv_t[0:D, 0:64] = wk_slice.T
        wkv_t[0:D, 64:128] = wv_slice.T
        wkv_t[D, 0:64] = bk_slice
        wkv_t[D, 64:128] = bv_slice

        wo_t = np.ascontiguousarray(wo_w[:, 512 * c : 512 * (c + 1)].T)  # [512, D]

        in_maps.append(
            dict(
                shared,
                wq=wq_t.astype(bf16),
                wkv=wkv_t.astype(bf16),
                wo=wo_t.astype(bf16),
            )
        )
    return in_maps


def _run(inputs, trace):
    global _COMPILED
    if _COMPILED is None:
        _COMPILED = _build()
    args = [
        np.asarray(inputs[k], dtype=np.float32)
        for k in (
            "x",
            "rope_cache",
            "wq_w",
            "wq_b",
            "wk_w",
            "wk_b",
            "wv_w",
            "wv_b",
            "wo_w",
        )
    ]
    in_maps = _prep_inputs(*args)
    res = run_bass_kernel_spmd(
        _COMPILED, in_maps, core_ids=list(range(N_CORES)), trace=trace
    )
    out = np.zeros((S, D), dtype=np.float32)
    for c in range(N_CORES):
        out += res.results[c]["partial"]
    out += np.asarray(inputs["wo_b"], np.float32)[None, :]
    return out.reshape(B, S, D).astype(np.float32), res


def kernel(**inputs):
    out, _ = _run(inputs, trace=False)
    return out


# expose the compiled-module runner for test harnesses that want tracing
def run_traced(**inputs):
    return _run(inputs, trace=True)


# revision 28
# speedup vs baseline: 1.2007x; 1.1927x over previous
"""Sliding-window GQA attention block (RoPE + QKV proj + SDPA + O proj) on 8
Trainium2 NeuronCores, head-sharded (1 kv-head group = 8 q-heads per core).

Contract: kernel(**inputs) takes the FULL unsharded inputs from
setup_inputs() and returns the FULL [1, 2048, 2880] output.

Per-core plan (core c owns q-heads [8c, 8c+8), kv-head c), all matmul
operands bf16:
  - QKV projections stream 512-wide bf16 matmuls (fp32 PSUM); RoPE in the
    PSUM epilogue (head-dim pre-permuted so rotate-half = partition-pair
    stream_shuffle, done in fp32 - bf16 shuffle is broken on HW). q lands in
    per-chunk tiles qc[c] [64, 8*512] (heads along free dim, order
    [0,2,4,6,1,3,5,7]), k in kT [64, S], v transposed to v_ext
    [seq, 64v + 64ones] tiles. x and wq stream via batched DMAs.
  - Attention per 128-query tile j, 4 heads per matmul (all 8 q-heads share
    the core's kv head): one [128,1024] PSUM tile per head-group holds
    scores vs key tiles j-1 | j; combined band-mask add (DVE), one exp
    (ACT, scale=1/8 folded) -> bf16 e tile. AV: lhsT = v_ext, rhs = e ->
    out.T [64 v + 64 dup-denominators, (h,q)]; denominators staged to SBUF
    (custom-DVE cannot read PSUM on HW), reciprocal_approx_fast, strided
    multiplies into per-tile aoj [128, 4*128] bf16.
  - Deep software pipeline: iteration j runs scores_j, AV_{j-1},
    oproj_{j-2} so no engine waits on a same-iteration producer.
  - O projection: 4x6 bf16 matmuls; PSUM chunks copied (ACT) into one
    [128, 2880] bf16 row tile, single DMA out per tile.
  - Host sums the 8 bf16 partials and adds wo_b.
"""
import sys

sys.path.insert(0, "/opt/trn_rl_repo")

import numpy as np

import concourse.bass as bass  # noqa: F401  (import keeps bass registered)
import concourse.tile as tile
from concourse import bacc, mybir
from concourse.bass_utils import run_bass_kernel_spmd

B, S, D = 1, 2048, 2880
H, KVH, HD = 64, 8, 64
WINDOW = 128
N_CORES = 8
DP = 2944  # padded contraction dim: 23 * 128 (2880 data + 1 ones row + pad)
KT = DP // 128  # 23 contraction tiles
NQT = S // 128  # 16 seq tiles
OCH = 480  # O-proj free chunk (6 * 480 = 2880)
WQG = [2, 7, 7, 7]  # wq k-tile DMA groups (small first group -> early PE start)
XH = [12, 11]  # x chunk half-tile k splits

F32 = mybir.dt.float32
BF16 = mybir.dt.bfloat16

# head order along q free dim: g0 = pair-firsts, g1 = pair-seconds
HEAD_ORDER = [0, 2, 4, 6, 1, 3, 5, 7]

# head-dim permutation: pairs (t, t+32) adjacent -> rotate-half partner is
# the neighbouring partition (stream_shuffle mask i^1 within quadrants)
PERM = np.empty(HD, dtype=np.int64)
PERM[0::2] = np.arange(32)
PERM[1::2] = np.arange(32) + 32

_COMPILED = None
_VERSION = 5  # bumped per kernel revision: busts stale compile/executable caches


def _build(debug=False):
    nc = bacc.Bacc("TRN2", target_bir_lowering=False, debug=False)

    ver_d = nc.dram_tensor(f"ver{_VERSION}", [1, 1], F32, kind="ExternalInput").ap()
    xT_d = nc.dram_tensor("xT", [DP, S], BF16, kind="ExternalInput").ap()
    wq_d = nc.dram_tensor("wq", [DP, 512], BF16, kind="ExternalInput").ap()
    wkv_d = nc.dram_tensor("wkv", [DP, 128], BF16, kind="ExternalInput").ap()
    wo_d = nc.dram_tensor("wo", [512, D], BF16, kind="ExternalInput").ap()
    cos_d = nc.dram_tensor("cosT", [128, S], F32, kind="ExternalInput").ap()
    sin_d = nc.dram_tensor("sinTs", [128, S], F32, kind="ExternalInput").ap()
    maAB_d = nc.dram_tensor("maAB", [128, 1024], F32, kind="ExternalInput").ap()
    id_d = nc.dram_tensor("id64", [64, 64], BF16, kind="ExternalInput").ap()
    out_d = nc.dram_tensor("partial", [S, D], BF16, kind="ExternalOutput").ap()
    if debug:
        dbg_q_d = nc.dram_tensor("dbg_q", [64, 8 * S], BF16, kind="ExternalOutput").ap()
        dbg_k_d = nc.dram_tensor("dbg_k", [64, S], BF16, kind="ExternalOutput").ap()
        dbg_v_d = nc.dram_tensor("dbg_v", [64, S], BF16, kind="ExternalOutput").ap()
        dbg_e_d = nc.dram_tensor("dbg_e", [128, 2048], BF16, kind="ExternalOutput").ap()

    Exp = mybir.ActivationFunctionType.Exp
    SHUF_MASK = [i ^ 1 for i in range(32)]

    # DRAM views with the 128-partition dim explicit, for batched DMAs
    xT_v = xT_d.rearrange("(k p) s -> p k s", p=128)  # [128, 23, 2048]
    wq_v = wq_d.rearrange("(k p) m -> p k m", p=128)  # [128, 23, 512]
    wkv_v = wkv_d.rearrange("(k p) m -> p k m", p=128)  # [128, 23, 128]

    with tile.TileContext(nc) as tc:
        with (
            tc.tile_pool(name="constp", bufs=1) as constp,
            tc.tile_pool(name="qkvp", bufs=1) as qkvp,
            tc.tile_pool(name="vextp", bufs=1) as vextp,
            tc.tile_pool(name="workp", bufs=2) as workp,
        ):
            cos_t = constp.tile([128, S], F32)
            sin_t = constp.tile([128, S], F32)
            maAB_t = constp.tile([128, 1024], F32)
            id_t = constp.tile([64, 64], BF16)
            ver_t = constp.tile([1, 1], F32)
            nc.sync.dma_start(ver_t[:], ver_d[:])

            # persistent activations: q per chunk, k/v full
            qc = [qkvp.tile([64, 8 * 512], BF16, name=f"qc{c}") for c in range(4)]
            kT = qkvp.tile([64, S], BF16, name="kT")
            vT = qkvp.tile([64, S], BF16, name="vT")
            wo_sb = [qkvp.tile([128, D], BF16, name=f"wo{t}") for t in range(4)]
            v_ext = [vextp.tile([128, 128], BF16, name=f"vx{i}") for i in range(NQT)]

            # ---------------- Phase 1: QKV projections + RoPE ----------------
            with (
                tc.tile_pool(name="wpool", bufs=1) as wpool,
                tc.tile_pool(name="xsp", bufs=4) as xsp,
                tc.tile_pool(name="psq", bufs=6, space="PSUM") as psq,
            ):
                wq_g = [
                    wpool.tile([128, 512 * n], BF16, name=f"wqg{g}")
                    for g, n in enumerate(WQG)
                ]
                wkv_t = wpool.tile([128, 128 * KT], BF16, name="wkv_t")
                # x chunks as half-tiles (k 0:12 | 12:23), 4-buf rotation so the
                # next chunk's first half streams while the current one computes
                xh = [
                    [
                        xsp.tile([128, 512 * n], BF16, name="xh", tag="xh")
                        for n in XH
                    ]
                    for _ in range(4)
                ]

                def dma_x(sq, h, k0, k1):
                    base = 0 if h == 0 else XH[0]
                    nc.sync.dma_start(
                        xh[sq][h].rearrange("p (k s) -> p k s", s=512)[
                            :, k0 - base : k1 - base, :
                        ],
                        xT_v[:, k0:k1, 512 * sq : 512 * (sq + 1)],
                    )

                # startup order: small wq group + wkv + first x slice first
                k0 = 0
                x0_parts = [[(0, 2)], [(2, 9)], [(9, 12), (12, 16)], [(16, 23)]]
                for g, n in enumerate(WQG):
                    nc.sync.dma_start(
                        wq_g[g].rearrange("p (k m) -> p k m", m=512),
                        wq_v[:, k0 : k0 + n, :],
                    )
                    if g == 0:
                        nc.sync.dma_start(
                            wkv_t.rearrange("p (k m) -> p k m", m=128), wkv_v
                        )
                    for a, b in x0_parts[g]:
                        dma_x(0, 0 if a < XH[0] else 1, a, b)
                    k0 += n
                nc.sync.dma_start(cos_t[:], cos_d[:])
                nc.sync.dma_start(sin_t[:], sin_d[:])
                nc.sync.dma_start(maAB_t[:], maAB_d[:])
                nc.sync.dma_start(id_t[:], id_d[:])
                for t in range(4):
                    nc.sync.dma_start(wo_sb[t][:], wo_d[128 * t : 128 * (t + 1), :])
                for sq in range(1, 4):
                    dma_x(sq, 0, 0, XH[0])
                    dma_x(sq, 1, XH[0], KT)

                def wq_slice(k, mt):
                    g, i = 0, k
                    for n in WQG:
                        if i < n:
                            break
                        g, i = g + 1, i - n
                    c = 512 * i + 128 * mt
                    return wq_g[g][:, c : c + 128]

                def x_slice(sq, k):
                    h = 0 if k < XH[0] else 1
                    i = k if h == 0 else k - XH[0]
                    return xh[sq][h][:, 512 * i : 512 * (i + 1)]

                def rope_epilogue(sq, mt, ps):
                    c0 = 512 * sq
                    # fp32 through the shuffle (bf16 stream_shuffle is
                    # broken on TRN2 hardware); bf16 from the muls onward
                    t_all = workp.tile([128, 512], F32, tag="ra", name="t_all")
                    nc.scalar.copy(t_all[:], ps[:])
                    t_shuf = workp.tile([128, 512], F32, tag="rb", name="t_shuf")
                    nc.vector.stream_shuffle(t_shuf[:], t_all[:], SHUF_MASK)
                    t_cos = workp.tile([128, 512], BF16, tag="rc", name="t_cos")
                    nc.vector.tensor_mul(t_cos[:], t_all[:], cos_t[:, c0 : c0 + 512])
                    t_sin = workp.tile([128, 512], BF16, tag="rd", name="t_sin")
                    nc.vector.tensor_mul(t_sin[:], t_shuf[:], sin_t[:, c0 : c0 + 512])
                    # psum halves -> adjacent head blocks of this chunk's q
                    # tile (host packs wq columns in HEAD_ORDER)
                    b0 = 512 * (2 * mt)
                    b1 = 512 * (2 * mt + 1)
                    nc.vector.tensor_add(
                        qc[sq][:, b0 : b0 + 512], t_cos[0:64, :], t_sin[0:64, :]
                    )
                    nc.vector.tensor_add(
                        qc[sq][:, b1 : b1 + 512], t_cos[64:128, :], t_sin[64:128, :]
                    )

                for sq in range(4):
                    c0 = 512 * sq
                    psums = [
                        psq.tile([128, 512], F32, name="psq_t", tag="psq_t")
                        for _ in range(5)
                    ]
                    # pairwise-interleaved accumulation: same-bank matmuls stay
                    # one apart (avoids the accumulation hazard) while early
                    # m-tiles finish mid-chunk so their epilogues drain on
                    # ACT/DVE during the rest of the accumulation
                    for grp in ((0, 1), (2, 3), (4,)):
                        for k in range(KT):
                            for mt in grp:
                                nc.tensor.matmul(
                                    psums[mt][:],
                                    wq_slice(k, mt)
                                    if mt < 4
                                    else wkv_t[:, 128 * k : 128 * (k + 1)],
                                    x_slice(sq, k),
                                    start=(k == 0),
                                    stop=(k == KT - 1),
                                )
                        for mt in grp:
                            if mt < 4:
                                rope_epilogue(sq, mt, psums[mt])
                    # kv epilogue: k rope (rows 0:64) + v copy (rows 64:128)
                    ps = psums[4]
                    t_allk = workp.tile([128, 512], F32, tag="ra", name="t_allk")
                    nc.scalar.copy(t_allk[0:64, :], ps[0:64, :])
                    t_shufk = workp.tile([128, 512], F32, tag="rb", name="t_shufk")
                    nc.vector.stream_shuffle(t_shufk[0:64, :], t_allk[0:64, :], SHUF_MASK)
                    t_cosk = workp.tile([128, 512], BF16, tag="rc", name="t_cosk")
                    nc.vector.tensor_mul(
                        t_cosk[0:64, :], t_allk[0:64, :], cos_t[0:64, c0 : c0 + 512]
                    )
                    t_sink = workp.tile([128, 512], BF16, tag="rd", name="t_sink")
                    nc.vector.tensor_mul(
                        t_sink[0:64, :], t_shufk[0:64, :], sin_t[0:64, c0 : c0 + 512]
                    )
                    nc.vector.tensor_add(
                        kT[:, c0 : c0 + 512], t_cosk[0:64, :], t_sink[0:64, :]
                    )
                    nc.vector.tensor_copy(vT[:, c0 : c0 + 512], ps[64:128, :])
                    # v transposes for this chunk's 4 seq tiles
                    for i in range(4 * sq, 4 * sq + 4):
                        tr = psq.tile([128, 64], BF16, name="vtr", tag="vtr", bufs=2)
                        nc.tensor.transpose(tr[:], vT[:, 128 * i : 128 * (i + 1)], id_t[:])
                        nc.vector.tensor_copy(v_ext[i][:, 0:64], tr[:])
                        nc.gpsimd.memset(v_ext[i][:, 64:128], 1.0)

            # ------------- Phase 2: attention + O-projection, pipelined -------
            with (
                tc.tile_pool(name="epool", bufs=6) as epool,
                tc.tile_pool(name="recp", bufs=2) as recp,
                tc.tile_pool(name="aop", bufs=4) as aop,
                tc.tile_pool(name="outsp", bufs=2) as outsp,
                tc.tile_pool(name="psS", bufs=2, space="PSUM") as psS,
                tc.tile_pool(name="psAV", bufs=1, space="PSUM") as psAV,
                tc.tile_pool(name="psP", bufs=2, space="PSUM") as psP,
            ):
                e_hist = {}
                ao_hist = {}

                def scores(j):
                    """Scores + mask + exp for tile j -> e_hist[j]."""
                    cqc = qc[j // 4].rearrange("p (b s) -> p b s", s=512)
                    jo = 128 * (j % 4)
                    e_g = []
                    for g in range(2):
                        sc = psS.tile([128, 1024], F32, name="sc", tag="ps_s")
                        qslc = cqc[:, 4 * g : 4 * g + 4, jo : jo + 128]
                        if j > 0:
                            nc.tensor.matmul(
                                sc[:, 0:512],
                                kT[:, 128 * (j - 1) : 128 * j],
                                qslc,
                                start=True,
                                stop=True,
                            )
                        nc.tensor.matmul(
                            sc[:, 512:1024],
                            kT[:, 128 * j : 128 * (j + 1)],
                            qslc,
                            start=True,
                            stop=True,
                        )
                        e_t = epool.tile([128, 1024], BF16, tag="e", name="e_t")
                        if j > 0:
                            nc.vector.tensor_add(sc[:], sc[:], maAB_t[:])
                            nc.scalar.activation(e_t[:], sc[:], Exp, scale=0.125)
                        else:
                            nc.vector.tensor_add(
                                sc[:, 512:1024], sc[:, 512:1024], maAB_t[:, 512:1024]
                            )
                            nc.scalar.activation(
                                e_t[:, 512:1024], sc[:, 512:1024], Exp, scale=0.125
                            )
                        e_g.append(e_t)
                    e_hist[j] = e_g

                def av_norm(j):
                    """AV + denominators + normalize for tile j -> ao_hist[j]."""
                    e_g = e_hist.pop(j)
                    pav = psAV.tile([128, 1024], F32, name="pav", tag="pav")
                    for g in range(2):
                        if j > 0:
                            nc.tensor.matmul(
                                pav[:, 512 * g : 512 * (g + 1)],
                                v_ext[j - 1][:],
                                e_g[g][:, 0:512],
                                start=True,
                                stop=False,
                            )
                        nc.tensor.matmul(
                            pav[:, 512 * g : 512 * (g + 1)],
                            v_ext[j][:],
                            e_g[g][:, 512:1024],
                            start=(j == 0),
                            stop=True,
                        )
                    # custom-DVE ops cannot read PSUM on HW: stage via ACT
                    den = recp.tile([64, 1024], F32, name="den", tag="den")
                    nc.scalar.copy(den[:], pav[64:128, :])
                    rec = recp.tile([64, 1024], F32, name="rec", tag="rec")
                    nc.vector.reciprocal_approx_fast(rec[:], den[:])
                    aoj = aop.tile([128, 512], BF16, tag="ao", name="aoj")
                    aov = aoj.rearrange("p (t s) -> p t s", t=4)
                    for g in range(2):
                        nc.vector.tensor_mul(
                            aov[64 * g : 64 * (g + 1), :, :],
                            pav[0:64, 512 * g : 512 * (g + 1)].rearrange(
                                "p (h q) -> p h q", h=4
                            ),
                            rec[:, 512 * g : 512 * (g + 1)].rearrange(
                                "p (h q) -> p h q", h=4
                            ),
                        )
                    ao_hist[j] = aoj

                def oproj(j):
                    aoj = ao_hist.pop(j)
                    out_row = outsp.tile([128, D], BF16, tag="orow", name="orow")
                    for ch in range(6):
                        pp = psP.tile([128, OCH], F32, name="pp", tag="pp")
                        for t in range(4):
                            nc.tensor.matmul(
                                pp[:],
                                aoj[:, 128 * t : 128 * (t + 1)],
                                wo_sb[t][:, OCH * ch : OCH * (ch + 1)],
                                start=(t == 0),
                                stop=(t == 3),
                            )
                        nc.scalar.copy(out_row[:, OCH * ch : OCH * (ch + 1)], pp[:])
                    nc.sync.dma_start(out_d[128 * j : 128 * (j + 1), :], out_row[:])

                for j in range(NQT):
                    scores(j)
                    if j >= 1:
                        av_norm(j - 1)
                    if j >= 2:
                        oproj(j - 2)
                    if debug and j == 2:
                        nc.sync.dma_start(dbg_e_d[:, 0:1024], e_hist[2][0][:])
                        nc.sync.dma_start(dbg_e_d[:, 1024:2048], e_hist[2][1][:])
                av_norm(NQT - 1)
                oproj(NQT - 2)
                oproj(NQT - 1)
                if debug:
                    for c in range(4):
                        nc.sync.dma_start(
                            dbg_q_d[:, 4096 * c : 4096 * (c + 1)], qc[c][:]
                        )
                    nc.sync.dma_start(dbg_k_d[:], kT[:])
                    nc.sync.dma_start(dbg_v_d[:], vT[:])
    nc.compile()
    return nc


def _prep_inputs(x, rope_cache, wq_w, wq_b, wk_w, wk_b, wv_w, wv_b, wo_w):
    """Build the shared + per-core input maps."""
    import ml_dtypes

    bf16 = ml_dtypes.bfloat16

    xT = np.zeros((DP, S), dtype=np.float32)
    xT[0:D, :] = np.ascontiguousarray(x[0].T)
    xT[D, :] = 1.0  # bias row

    cos = np.asarray(rope_cache[:, 0, :], dtype=np.float32)  # [S, 64]
    sin = np.asarray(rope_cache[:, 1, :], dtype=np.float32)
    cosP = cos[:, PERM].T  # [64, S] permuted head-dim rows
    sinP = sin[:, PERM].T
    sign = np.where(PERM < 32, -1.0, 1.0).astype(np.float32)[:, None]
    sinPs = sinP * sign
    cosT = np.concatenate([cosP, cosP], axis=0).astype(np.float32)  # [128, S]
    sinTs = np.concatenate([sinPs, sinPs], axis=0).astype(np.float32)

    kk = np.arange(128)[:, None]
    qq = np.arange(128)[None, :]
    maB1 = np.where(kk <= qq, 0.0, -1e30).astype(np.float32)  # same-tile causal
    maA1 = np.where(qq < kk, 0.0, -1e30).astype(np.float32)  # prev-tile window
    maAB = np.concatenate([np.tile(maA1, (1, 4)), np.tile(maB1, (1, 4))], axis=1)

    id64 = np.eye(64, dtype=np.float32).astype(bf16)

    shared = dict(
        xT=xT.astype(bf16),
        cosT=cosT,
        sinTs=sinTs,
        maAB=maAB,
        id64=id64,
    )
    shared[f"ver{_VERSION}"] = np.zeros((1, 1), np.float32)

    in_maps = []
    for c in range(N_CORES):
        # wq slice: q heads [8c, 8c+8) in block order HEAD_ORDER, head-dim
        # permuted, transposed, bias row
        wq_rows = []
        bq_rows = []
        for lh in HEAD_ORDER:
            g = 8 * c + lh
            wq_rows.append(wq_w[64 * g + PERM, :])  # [64, D]
            bq_rows.append(wq_b[64 * g + PERM])
        wq_slice = np.concatenate(wq_rows, axis=0)  # [512, D]
        bq_slice = np.concatenate(bq_rows, axis=0)  # [512]
        wq_t = np.zeros((DP, 512), dtype=np.float32)
        wq_t[0:D, :] = wq_slice.T
        wq_t[D, :] = bq_slice

        wk_slice = wk_w[64 * c + PERM, :]  # [64, D] permuted
        bk_slice = wk_b[64 * c + PERM]
        wv_slice = wv_w[64 * c : 64 * (c + 1), :]  # unpermuted
        bv_slice = wv_b[64 * c : 64 * (c + 1)]
        wkv_t = np.zeros((DP, 128), dtype=np.float32)
        wkv_t[0:D, 0:64] = wk_slice.T
        wkv_t[0:D, 64:128] = wv_slice.T
        wkv_t[D, 0:64] = bk_slice
        wkv_t[D, 64:128] = bv_slice

        wo_t = np.ascontiguousarray(wo_w[:, 512 * c : 512 * (c + 1)].T)  # [512, D]

        in_maps.append(
            dict(
                shared,
                wq=wq_t.astype(bf16),
                wkv=wkv_t.astype(bf16),
                wo=wo_t.astype(bf16),
            )
        )
    return in_maps


def _run(inputs, trace):
    global _COMPILED
    if _COMPILED is None:
        _COMPILED = _build()
    args = [
        np.asarray(inputs[k], dtype=np.float32)
        for k in (
            "x",
            "rope_cache",
            "wq_w",
            "wq_b",
            "wk_w",
            "wk_b",
            "wv_w",
            "wv_b",
            "wo_w",
        )
    ]
    in_maps = _prep_inputs(*args)
    res = run_bass_kernel_spmd(
        _COMPILED, in_maps, core_ids=list(range(N_CORES)), trace=trace
    )
    out = np.zeros((S, D), dtype=np.float32)
    for c in range(N_CORES):
        out += res.results[c]["partial"]
    out += np.asarray(inputs["wo_b"], np.float32)[None, :]
    return out.reshape(B, S, D).astype(np.float32), res


def kernel(**inputs):
    out, _ = _run(inputs, trace=False)
    return out


# expose the compiled-module runner for test harnesses that want tracing
def run_traced(**inputs):
    return _run(inputs, trace=True)
